# revision 34
# baseline (speedup 1.0000x reference)
"""Linear attention (B=2, L=4096, DM=1024, H=16) on 8 trn2 NeuronCores.

Sharding: rows (B*L) split 8 ways -> each core owns 512 rows of each batch
(1024 rows total). Projections, feature map, denominators, V@S and the output
projection are all row-local. The only cross-core term is S = K^T Q per
(batch, head) -- reduced with one bf16 AllReduce of [128, 1024] (256 KB).

All matmuls run in bfloat16 (1 cycle/row at any free size on the PE, vs
float32r's 4x penalty below 256), activations/psums accumulate in fp32.

Layouts (matmul contracts over the partition dim; computes lhsT.T @ rhs):
 - host pre-transposes + casts activations to bf16: XT = X_c^T  [dm, l]
 - xt/W live in single [128, 8*1024] SBUF tiles filled by 2 big DMAs each
   (col = kc*1024 + j holds element [kc*128 + p, j]); ~40 DMAs total per
   core keeps the serial HWDGE/SP descriptor path (~625 ns per DMA) short.
 - q, k natural [l, d]: lhsT = xt chunk, rhs = W chunk; ELU+1 on DVE+Act.
 - vT [d, m]: lhsT = Wv chunk, rhs = xt chunk (bias via K=1 matmul).
 - S[b*64+e, g*512+i*64+d] packed in 2 psum banks; AllReduce in bf16 fires
   right after the S matmuls (~40 us, hidden behind the vT projection).
 - denominators: DVE mul/reduce interleaved with the k copyouts, transposed
   to recipT [16, ROWS] on the PE, reciprocal on DVE; per-(t) broadcast
   tiles rb[p, m] = recip[2t + (p>=64), m] built with PE selector matmuls
   (E_t.T @ recipT) during the AllReduce window -- partition broadcasts and
   per-row DMAs are both unavailable/slow here.
 - after the AllReduce, cc_out is reloaded once into two SBUF tiles (ccJ0
   at partitions 0-63, ccJ1 at 64-127) so each attnT matmul reads its
   [64, 64] S block directly: attnT[t] psum half j <- ccJ_j slice.T @ vT,
   then one DVE mul applies the reciprocal during copyout.
 - out_dense: lhsT = attnT chunk (stationary), rhs = Wo chunk; bias via
   K=1 matmul; stores as 8 [128, 1024] fp32 DMAs.
"""
import sys

sys.path.insert(0, "/opt/trn_rl_repo")
import numpy as np
import ml_dtypes

B, L, DM, H = 2, 4096, 1024, 16
D = DM // H  # 64
N_CORES = 8
ROWS = B * L // N_CORES  # 1024 rows per core
RPB = ROWS // B  # 512 rows per batch per core
NT = ROWS // 128  # 8 l-tiles per core (4 per batch)
KC = DM // 128  # 8 contraction chunks

_CACHE = {}


def _build():
    import concourse.bass as bass
    import concourse.mybir as mybir
    import concourse.tile as tile
    from concourse import bacc
    from concourse.masks import make_identity

    dt = mybir.dt
    f32, f32r, bf16 = dt.float32, dt.float32r, dt.bfloat16
    AFT = mybir.ActivationFunctionType

    nc = bacc.Bacc("TRN2", target_bir_lowering=False, debug=False,
                   num_devices=N_CORES)

    qT_d = nc.dram_tensor("qT", [DM, ROWS], bf16, kind="ExternalInput").ap()
    kT_d = nc.dram_tensor("kT", [DM, ROWS], bf16, kind="ExternalInput").ap()
    vT_d = nc.dram_tensor("vT", [DM, ROWS], bf16, kind="ExternalInput").ap()
    W_d = {w: nc.dram_tensor(w, [DM, DM], bf16, kind="ExternalInput").ap()
           for w in ("Wq", "Wk", "Wv", "Wo")}
    b_d = {b: nc.dram_tensor(b, [1, DM], bf16, kind="ExternalInput").ap()
           for b in ("bq", "bk", "bo")}
    bvT_d = nc.dram_tensor("bvT", [128, KC], f32, kind="ExternalInput").ap()
    E_d = nc.dram_tensor("Econst", [16, DM], bf16, kind="ExternalInput").ap()
    out_d = nc.dram_tensor("out", [ROWS, DM], f32, kind="ExternalOutput").ap()

    def big3(ap):
        # [DM, N] dram -> [128, KC, N] AP matching an [128, KC*N] SBUF tile
        return ap.rearrange("(kc p) n -> kc p n", p=128).transpose([1, 0, 2])

    with tile.TileContext(nc) as tc:
        with (
            tc.tile_pool(name="xt", bufs=1) as xt_pool,
            tc.tile_pool(name="w", bufs=1) as w_pool,
            tc.tile_pool(name="act", bufs=1) as act_pool,
            tc.tile_pool(name="tmp", bufs=3) as tmp_pool,
            tc.tile_pool(name="small", bufs=1) as small_pool,
            tc.tile_pool(name="ps", bufs=8, space="PSUM") as ps_pool,
            tc.tile_pool(name="dram", bufs=1, space="DRAM") as dram_pool,
        ):
            ones = small_pool.tile([1, 512], bf16, tag="ones", name="ones")
            nc.vector.memset(ones[:], 1.0)
            ident = small_pool.tile([128, 128], f32, tag="ident", name="ident")
            make_identity(nc, ident[:])
            Et = small_pool.tile([16, DM], bf16, tag="E", name="E")
            nc.sync.dma_start(Et[:], E_d)

            def load_bias(b):
                t = small_pool.tile([1, DM], bf16, tag="bias", name="bias",
                                    bufs=2)
                nc.sync.dma_start(t[:], b_d[b])
                return t

            def load_big(dram_ap, pool, tag, nsplit=2, interleave=None):
                """interleave: second (dram_ap, pool, tag) loaded with its
                splits alternating with this one's (so the kc-ordered
                consumers of both tiles see chunks arrive in step)."""
                srcs = [(dram_ap, pool.tile([128, KC * 1024], bf16, tag=tag,
                                            name=tag))]
                if interleave is not None:
                    ap2, pool2, tag2 = interleave
                    srcs.append((ap2, pool2.tile([128, KC * 1024], bf16,
                                                 tag=tag2, name=tag2)))
                step = KC // nsplit
                for s in range(nsplit):
                    for ap, t in srcs:
                        src = big3(ap)
                        dst = t[:].rearrange("p (kc n) -> p kc n", kc=KC)
                        nc.sync.dma_start(dst[:, s * step:(s + 1) * step, :],
                                          src[:, s * step:(s + 1) * step, :])
                if interleave is not None:
                    return srcs[0][1], srcs[1][1]
                return srcs[0][1]

            # ================= q/k projections =================
            # psum groups of 4 m-tiles (one batch): group g+1's banks are
            # disjoint from group g's (ring of 8), so copyouts drain behind
            # the next group's matmuls -- no boundary stall, p-state hot.
            def proj_half(xt, wt, bias, outs, mh, first=False):
                """outs[m][l, d] = elu(X @ W + b)+1 for m-tiles of batch mh.
                first=True: the n=0 group takes its bias matmul LAST, so the
                kernel's very first matmuls depend only on the leading
                xt/W DMA chunks, not on the bias load."""
                ms = [mh * 4 + i for i in range(4)]
                for n in range(2):
                    bias_last = first and n == 0
                    psums = {m: ps_pool.tile([128, 512], f32, tag="pp",
                                             name="pp") for m in ms}
                    if not bias_last:
                        for m in ms:
                            nc.tensor.matmul(psums[m][:], ones[:1, :128],
                                             bias[:1, n * 512:(n + 1) * 512],
                                             start=True, stop=False)
                    for kc in range(KC):
                        for m in ms:
                            nc.tensor.matmul(
                                psums[m][:],
                                xt[:, kc * 1024 + m * 128:
                                   kc * 1024 + (m + 1) * 128],
                                wt[:, kc * 1024 + n * 512:
                                   kc * 1024 + (n + 1) * 512],
                                start=(bias_last and kc == 0),
                                stop=(not bias_last and kc == KC - 1))
                    if bias_last:
                        for m in ms:
                            nc.tensor.matmul(psums[m][:], ones[:1, :128],
                                             bias[:1, n * 512:(n + 1) * 512],
                                             start=False, stop=True)
                    for m in ms:
                        mn = tmp_pool.tile([128, 512], f32, tag="mn",
                                           name="mn")
                        ex = tmp_pool.tile([128, 512], f32, tag="ex",
                                           name="ex")
                        nc.vector.tensor_scalar_min(mn[:], psums[m][:], 0.0)
                        nc.scalar.activation(ex[:], mn[:], AFT.Exp)
                        nc.vector.scalar_tensor_tensor(
                            outs[m][n][:], psums[m][:], 0.0, ex[:],
                            op0=mybir.AluOpType.max,
                            op1=mybir.AluOpType.add)

            def s_partial(b):
                """S partial for batch b, packed as head-pair blocks:
                S_ps[g][:, i*128:+128] = [k_h0|k_h1]^T [q_h0|q_h1] for the
                pair t = g*4+i (h0=2t). Half the matmuls of per-head S; the
                off-diagonal quarters are waste, the diagonal quarters are
                extracted by strided copies in launch_ar."""
                S_ps = [ps_pool.tile([128, 512], f32, tag="pp", name="S_ps")
                        for _ in range(2)]
                for g in range(2):
                    for i in range(4):
                        c0 = 2 * i * 64
                        for lc in range(NT // B):
                            m = b * (NT // B) + lc
                            nc.tensor.matmul(
                                S_ps[g][:, i * 128:(i + 1) * 128],
                                k_t[m][g][:, c0:c0 + 128],
                                q_t[m][g][:, c0:c0 + 128],
                                start=(lc == 0), stop=(lc == NT // B - 1))
                return S_ps

            def launch_ar(b, S_ps):
                """Extract diagonal quarters (strided), fire the AllReduce.
                ccst rows 0:64 = even heads' S, rows 64:128 = odd heads'."""
                ccst = small_pool.tile([128, 512], bf16, tag=f"ccst{b}",
                                       name="ccst")
                for g in range(2):
                    for j in range(2):
                        src = S_ps[g][j * 64:(j + 1) * 64, :].rearrange(
                            "p (i c) -> p i c", i=4)[:, :, j * 64:(j + 1) * 64]
                        dst = ccst[j * 64:(j + 1) * 64,
                                   g * 256:(g + 1) * 256].rearrange(
                            "p (i d) -> p i d", i=4)
                        nc.vector.tensor_copy(dst, src)
                cc_in = dram_pool.tile([128, 512], bf16, tag=f"ccin{b}",
                                       name="ccin")
                cc_out = dram_pool.tile([128, 512], bf16, tag=f"ccout{b}",
                                        name="ccout")
                nc.sync.dma_start(cc_in[:], ccst[:])
                nc.gpsimd.collective_compute(
                    "AllReduce", mybir.AluOpType.add,
                    replica_groups=[list(range(N_CORES))],
                    ins=[cc_in[:].opt()], outs=[cc_out[:].opt()])
                return cc_out

            bias_q = load_bias("bq")
            bias_k = load_bias("bk")
            xt_q, w_q = load_big(qT_d, xt_pool, "xtA", nsplit=8,
                                 interleave=(W_d["Wq"], w_pool, "wA"))
            xt_k, w_k = load_big(kT_d, xt_pool, "xtB", nsplit=8,
                                 interleave=(W_d["Wk"], w_pool, "wB"))

            # q/k feature tiles are split per n-half: the S matmuls for
            # head group g read only half g, and per-half tiles avoid a
            # false wait on the other half's ELU chain (dep tracking is
            # tile-granular)
            q_t = [[act_pool.tile([128, 512], bf16, tag=f"q{m}h{h}",
                                  name=f"q{m}h{h}") for h in range(2)]
                   for m in range(NT)]
            k_t = [[act_pool.tile([128, 512], bf16, tag=f"k{m}h{h}",
                                  name=f"k{m}h{h}") for h in range(2)]
                   for m in range(NT)]

            # batch 0: project, S partial, fire AllReduce 0 early
            proj_half(xt_q, w_q, bias_q, q_t, 0)
            proj_half(xt_k, w_k, bias_k, k_t, 0)
            cc0 = launch_ar(0, s_partial(0))

            # vT/out-proj loads: after the cc_in(0) DMA (SP is in-order; the
            # xt_v WAR wait on xtA would otherwise delay the collective),
            # before cc_in(1) so they issue as soon as the q tiles free.
            xt_v = load_big(vT_d, xt_pool, "xtA", nsplit=2)
            w_v = load_big(W_d["Wv"], w_pool, "wC", nsplit=2)
            bvT = small_pool.tile([128, KC], f32, tag="bvT", name="bvT")
            nc.sync.dma_start(bvT[:], bvT_d)
            w_o = load_big(W_d["Wo"], w_pool, "wA", nsplit=2)
            bias_o = load_bias("bo")

            # batch 1: project, S partial, AllReduce 1 (queues behind AR 0)
            proj_half(xt_q, w_q, bias_q, q_t, 1)
            proj_half(xt_k, w_k, bias_k, k_t, 1)
            cc1 = launch_ar(1, s_partial(1))

            # ---- denominators: den[m][l, h] = sum_d q*k on DVE (emitted
            # after the S staging so the AllReduces are not stuck behind
            # them in the in-order DVE queue) ----
            dens = []
            for m in range(NT):
                den = tmp_pool.tile([128, 16], f32, tag="den", name="den",
                                    bufs=NT)
                for half in range(2):
                    prod = tmp_pool.tile([128, 512], bf16, tag="prod",
                                         name="prod")
                    nc.vector.tensor_mul(
                        prod[:], q_t[m][half][:], k_t[m][half][:])
                    nc.vector.reduce_sum(
                        den[:, half * 8:(half + 1) * 8],
                        prod[:].rearrange("p (h d) -> p h d", h=8),
                        axis=mybir.AxisListType.X)
                dens.append(den)

            # ---- vT projection overlaps the AllReduces ----
            recipT = small_pool.tile([16, ROWS], f32, tag="recipT",
                                     name="recipT")
            recipT_r = small_pool.tile([16, ROWS], bf16, tag="recipTr",
                                       name="recipTr")
            rbs = [act_pool.tile([128, ROWS], bf16, tag=f"rb{t}",
                                 name=f"rb{t}") for t in range(KC)]

            def dent_half(b):
                # transpose dens -> recipT cols, reciprocal, bf16 copy
                for m in range(b * 4, b * 4 + 4):
                    dent = ps_pool.tile([16, 128], f32, tag="pp",
                                        name="dent")
                    nc.tensor.transpose(dent[:], dens[m][:], ident[:])
                    nc.vector.tensor_scalar_add(
                        recipT[:, m * 128:(m + 1) * 128], dent[:], 1e-6)
                sl = slice(b * 512, (b + 1) * 512)
                nc.vector.reciprocal(recipT[:, sl], recipT[:, sl])
                nc.vector.tensor_copy(recipT_r[:, sl], recipT[:, sl])

            def rb_half(b):
                # rb[t][p, b-half] = recip[2t + (p>=64)] via selector matmul
                for t in range(KC):
                    psr = ps_pool.tile([128, 512], f32, tag="pp", name="psr")
                    nc.tensor.matmul(psr[:], Et[:, t * 128:(t + 1) * 128],
                                     recipT_r[:, b * 512:(b + 1) * 512],
                                     start=True, stop=True)
                    nc.scalar.activation(rbs[t][:, b * 512:(b + 1) * 512],
                                         psr[:], AFT.Copy)

            vTs = [act_pool.tile([128, ROWS], bf16, tag=f"vt{t}",
                                 name=f"vt{t}")
                   for t in range(KC)]
            for t in range(KC):
                ps2 = [ps_pool.tile([128, 512], f32, tag="pp", name="pp")
                       for _ in range(2)]
                for kc in range(KC):
                    for n in range(2):
                        nc.tensor.matmul(
                            ps2[n][:],
                            w_v[:, kc * 1024 + t * 128:
                                kc * 1024 + (t + 1) * 128],
                            xt_v[:, kc * 1024 + n * 512:
                                 kc * 1024 + (n + 1) * 512],
                            start=(kc == 0), stop=(kc == KC - 1))
                if t == 3:
                    dent_half(0)
                elif t == 5:
                    dent_half(1)
                elif t == 6:
                    rb_half(0)
                for n in range(2):
                    # bias bv is per-partition here (rows = dm): fuse it
                    # into the copyout on the Act engine
                    nc.scalar.activation(
                        vTs[t][:, n * 512:(n + 1) * 512], ps2[n][:],
                        AFT.Identity, bias=bvT[:, t:t + 1])

            # ---- per batch: reload reduced S, attnT, output projection ----
            attnT = [act_pool.tile([128, ROWS], bf16, tag=f"at{t}",
                                   name=f"attnT{t}")
                     for t in range(KC)]

            def ccj_load(cc_out, b):
                ccJ0 = small_pool.tile([64, 512], bf16, tag=f"ccJ0{b}",
                                       name="ccJ0")
                ccJ1 = small_pool.tile([128, 512], bf16, tag=f"ccJ1{b}",
                                       name="ccJ1")
                nc.sync.dma_start(ccJ0[0:64, :], cc_out[0:64, :])
                nc.sync.dma_start(ccJ1[64:128, :], cc_out[64:128, :])
                return ccJ0, ccJ1

            def attn_half(ccJs, b):
                for t in range(KC):
                    ps = ps_pool.tile([128, 512], f32, tag="pp", name="pa")
                    for j in range(2):
                        col = (t // 4) * 256 + (t % 4) * 64
                        nc.tensor.matmul(
                            ps[j * 64:(j + 1) * 64, :],
                            ccJs[j][j * 64:(j + 1) * 64, col:col + 64],
                            vTs[t][j * 64:(j + 1) * 64,
                                   b * RPB:(b + 1) * RPB],
                            start=True, stop=True)
                    nc.vector.tensor_mul(
                        attnT[t][:, b * RPB:(b + 1) * RPB], ps[:],
                        rbs[t][:, b * RPB:(b + 1) * RPB])

            def out_half(mh, mid_hook=None, taper=False):
                base = mh * 4
                for n in range(2):
                    if n == 1 and mid_hook is not None:
                        mid_hook()
                    # taper the very last groups (2,1,1) so the drain tail
                    # is one short copyout+store deep instead of four
                    if taper and n == 1:
                        grps = [[base], [base + 1], [base + 2], [base + 3]]
                    else:
                        grps = [[base + i for i in range(4)]]
                    for ms in grps:
                        psums = {m: ps_pool.tile([128, 512], f32, tag="pp",
                                                 name="pp") for m in ms}
                        for m in ms:
                            nc.tensor.matmul(psums[m][:], ones[:1, :128],
                                             bias_o[:1,
                                                    n * 512:(n + 1) * 512],
                                             start=True, stop=False)
                        for kc in range(KC):
                            for m in ms:
                                nc.tensor.matmul(
                                    psums[m][:],
                                    attnT[kc][:, m * 128:(m + 1) * 128],
                                    w_o[:, kc * 1024 + n * 512:
                                        kc * 1024 + (n + 1) * 512],
                                    start=False, stop=(kc == KC - 1))
                        for m in ms:
                            ot = tmp_pool.tile([128, 512], f32,
                                               tag=("mn" if m % 2 else "ex"),
                                               name="ot")
                            if m % 2:
                                nc.scalar.activation(ot[:], psums[m][:],
                                                     AFT.Copy)
                            else:
                                nc.vector.tensor_copy(ot[:], psums[m][:])
                            nc.sync.dma_start(
                                out_d[m * 128:(m + 1) * 128,
                                      n * 512:(n + 1) * 512], ot[:])

            ccJs0 = ccj_load(cc0, 0)
            attn_half(ccJs0, 0)
            rb_half(1)
            # ccJ(1) DMAs are emitted between out(0)'s two store groups:
            # SP is in-order, so putting them after all out(0) stores would
            # delay them to ~the last store, stalling attnT(1); putting them
            # before would park SP on the AllReduce-1 semaphore and stall
            # the early stores instead.
            ccJs1 = []
            out_half(0, mid_hook=lambda: ccJs1.extend(ccj_load(cc1, 1)))
            attn_half(ccJs1, 1)
            out_half(1, taper=True)

    nc.compile()
    return nc


def _get_nc():
    if "nc" not in _CACHE:
        _CACHE["nc"] = _build()
    return _CACHE["nc"]


def _make_econst():
    E = np.zeros((16, DM), np.float32)
    for t in range(KC):
        E[2 * t, t * 128:t * 128 + 64] = 1.0
        E[2 * t + 1, t * 128 + 64:(t + 1) * 128] = 1.0
    return E


def kernel(query, key, value, Wq, bq, Wk, bk, Wv, bv, Wo, bo, **kw):
    from concourse.bass_utils import run_bass_kernel_spmd

    nc = _get_nc()
    bf = ml_dtypes.bfloat16
    query = np.asarray(query, dtype=np.float32)
    key = np.asarray(key, dtype=np.float32)
    value = np.asarray(value, dtype=np.float32)
    weights = {n: np.ascontiguousarray(np.asarray(w, np.float32).astype(bf))
               for n, w in (("Wq", Wq), ("Wk", Wk), ("Wv", Wv), ("Wo", Wo))}
    biases = {n: np.ascontiguousarray(
                  np.asarray(b, np.float32).reshape(1, DM).astype(bf))
              for n, b in (("bq", bq), ("bk", bk), ("bo", bo))}
    biases["bvT"] = np.ascontiguousarray(
        np.asarray(bv, np.float32).reshape(KC, 128).T)
    econst = _make_econst()

    in_maps = []
    for c in range(N_CORES):
        sl = slice(c * RPB, (c + 1) * RPB)
        m = {
            "qT": np.ascontiguousarray(
                np.concatenate([query[b, sl] for b in range(B)], 0).T
            ).astype(bf),
            "kT": np.ascontiguousarray(
                np.concatenate([key[b, sl] for b in range(B)], 0).T
            ).astype(bf),
            "vT": np.ascontiguousarray(
                np.concatenate([value[b, sl] for b in range(B)], 0).T
            ).astype(bf),
            "Econst": econst.astype(bf),
        }
        m.update(weights)
        m.update(biases)
        in_maps.append(m)

    res = run_bass_kernel_spmd(nc, in_maps, list(range(N_CORES)), **kw)
    out = np.empty((B, L, DM), np.float32)
    for c in range(N_CORES):
        o = np.asarray(res.results[c]["out"]).astype(np.float32)
        for b in range(B):
            out[b, c * RPB:(c + 1) * RPB] = o[b * RPB:(b + 1) * RPB]
    if kw:
        return out, res
    return out


# revision 35
# speedup vs baseline: 1.0016x; 1.0016x over previous
"""Linear attention (B=2, L=4096, DM=1024, H=16) on 8 trn2 NeuronCores.

Sharding: rows (B*L) split 8 ways -> each core owns 512 rows of each batch
(1024 rows total). Projections, feature map, denominators, V@S and the output
projection are all row-local. The only cross-core term is S = K^T Q per
(batch, head) -- reduced with one bf16 AllReduce of [128, 1024] (256 KB).

All matmuls run in bfloat16 (1 cycle/row at any free size on the PE, vs
float32r's 4x penalty below 256), activations/psums accumulate in fp32.

Layouts (matmul contracts over the partition dim; computes lhsT.T @ rhs):
 - host pre-transposes + casts activations to bf16: XT = X_c^T  [dm, l]
 - xt/W live in single [128, 8*1024] SBUF tiles filled by 2 big DMAs each
   (col = kc*1024 + j holds element [kc*128 + p, j]); ~40 DMAs total per
   core keeps the serial HWDGE/SP descriptor path (~625 ns per DMA) short.
 - q, k natural [l, d]: lhsT = xt chunk, rhs = W chunk; ELU+1 on DVE+Act.
 - vT [d, m]: lhsT = Wv chunk, rhs = xt chunk (bias via K=1 matmul).
 - S[b*64+e, g*512+i*64+d] packed in 2 psum banks; AllReduce in bf16 fires
   right after the S matmuls (~40 us, hidden behind the vT projection).
 - denominators: DVE mul/reduce interleaved with the k copyouts, transposed
   to recipT [16, ROWS] on the PE, reciprocal on DVE; per-(t) broadcast
   tiles rb[p, m] = recip[2t + (p>=64), m] built with PE selector matmuls
   (E_t.T @ recipT) during the AllReduce window -- partition broadcasts and
   per-row DMAs are both unavailable/slow here.
 - after the AllReduce, cc_out is reloaded once into two SBUF tiles (ccJ0
   at partitions 0-63, ccJ1 at 64-127) so each attnT matmul reads its
   [64, 64] S block directly: attnT[t] psum half j <- ccJ_j slice.T @ vT,
   then one DVE mul applies the reciprocal during copyout.
 - out_dense: lhsT = attnT chunk (stationary), rhs = Wo chunk; bias via
   K=1 matmul; stores as 8 [128, 1024] fp32 DMAs.
"""
import sys

sys.path.insert(0, "/opt/trn_rl_repo")
import numpy as np
import ml_dtypes

B, L, DM, H = 2, 4096, 1024, 16
D = DM // H  # 64
N_CORES = 8
ROWS = B * L // N_CORES  # 1024 rows per core
RPB = ROWS // B  # 512 rows per batch per core
NT = ROWS // 128  # 8 l-tiles per core (4 per batch)
KC = DM // 128  # 8 contraction chunks

_CACHE = {}


def _build():
    import concourse.bass as bass
    import concourse.mybir as mybir
    import concourse.tile as tile
    from concourse import bacc
    from concourse.masks import make_identity

    dt = mybir.dt
    f32, f32r, bf16 = dt.float32, dt.float32r, dt.bfloat16
    AFT = mybir.ActivationFunctionType

    nc = bacc.Bacc("TRN2", target_bir_lowering=False, debug=False,
                   num_devices=N_CORES)

    qT_d = nc.dram_tensor("qT", [DM, ROWS], bf16, kind="ExternalInput").ap()
    kT_d = nc.dram_tensor("kT", [DM, ROWS], bf16, kind="ExternalInput").ap()
    vT_d = nc.dram_tensor("vT", [DM, ROWS], bf16, kind="ExternalInput").ap()
    W_d = {w: nc.dram_tensor(w, [DM, DM], bf16, kind="ExternalInput").ap()
           for w in ("Wq", "Wk", "Wv", "Wo")}
    b_d = {b: nc.dram_tensor(b, [1, DM], bf16, kind="ExternalInput").ap()
           for b in ("bq", "bk", "bo")}
    bvT_d = nc.dram_tensor("bvT", [128, KC], f32, kind="ExternalInput").ap()
    E_d = nc.dram_tensor("Econst", [16, DM], bf16, kind="ExternalInput").ap()
    out_d = nc.dram_tensor("out", [ROWS, DM], f32, kind="ExternalOutput").ap()

    def big3(ap):
        # [DM, N] dram -> [128, KC, N] AP matching an [128, KC*N] SBUF tile
        return ap.rearrange("(kc p) n -> kc p n", p=128).transpose([1, 0, 2])

    with tile.TileContext(nc) as tc:
        with (
            tc.tile_pool(name="xt", bufs=1) as xt_pool,
            tc.tile_pool(name="w", bufs=1) as w_pool,
            tc.tile_pool(name="act", bufs=1) as act_pool,
            tc.tile_pool(name="tmp", bufs=3) as tmp_pool,
            tc.tile_pool(name="small", bufs=1) as small_pool,
            tc.tile_pool(name="ps", bufs=8, space="PSUM") as ps_pool,
            tc.tile_pool(name="dram", bufs=1, space="DRAM") as dram_pool,
        ):
            ones = small_pool.tile([1, 512], bf16, tag="ones", name="ones")
            nc.vector.memset(ones[:], 1.0)
            ident = small_pool.tile([128, 128], f32, tag="ident", name="ident")
            make_identity(nc, ident[:])
            Et = small_pool.tile([16, DM], bf16, tag="E", name="E")
            nc.sync.dma_start(Et[:], E_d)

            def load_bias(b):
                t = small_pool.tile([1, DM], bf16, tag="bias", name="bias",
                                    bufs=2)
                nc.sync.dma_start(t[:], b_d[b])
                return t

            def load_big(dram_ap, pool, tag, nsplit=2, interleave=None):
                """interleave: second (dram_ap, pool, tag) loaded with its
                splits alternating with this one's (so the kc-ordered
                consumers of both tiles see chunks arrive in step)."""
                srcs = [(dram_ap, pool.tile([128, KC * 1024], bf16, tag=tag,
                                            name=tag))]
                if interleave is not None:
                    ap2, pool2, tag2 = interleave
                    srcs.append((ap2, pool2.tile([128, KC * 1024], bf16,
                                                 tag=tag2, name=tag2)))
                step = KC // nsplit
                for s in range(nsplit):
                    for ap, t in srcs:
                        src = big3(ap)
                        dst = t[:].rearrange("p (kc n) -> p kc n", kc=KC)
                        nc.sync.dma_start(dst[:, s * step:(s + 1) * step, :],
                                          src[:, s * step:(s + 1) * step, :])
                if interleave is not None:
                    return srcs[0][1], srcs[1][1]
                return srcs[0][1]

            # ================= q/k projections =================
            # psum groups of 4 m-tiles (one batch): group g+1's banks are
            # disjoint from group g's (ring of 8), so copyouts drain behind
            # the next group's matmuls -- no boundary stall, p-state hot.
            def proj_half(xt, wt, bias, outs, mh, first=False):
                """outs[m][l, d] = elu(X @ W + b)+1 for m-tiles of batch mh.
                first=True: the n=0 group takes its bias matmul LAST, so the
                kernel's very first matmuls depend only on the leading
                xt/W DMA chunks, not on the bias load."""
                ms = [mh * 4 + i for i in range(4)]
                for n in range(2):
                    bias_last = first and n == 0
                    psums = {m: ps_pool.tile([128, 512], f32, tag="pp",
                                             name="pp") for m in ms}
                    if not bias_last:
                        for m in ms:
                            nc.tensor.matmul(psums[m][:], ones[:1, :128],
                                             bias[:1, n * 512:(n + 1) * 512],
                                             start=True, stop=False)
                    for kc in range(KC):
                        for m in ms:
                            nc.tensor.matmul(
                                psums[m][:],
                                xt[:, kc * 1024 + m * 128:
                                   kc * 1024 + (m + 1) * 128],
                                wt[:, kc * 1024 + n * 512:
                                   kc * 1024 + (n + 1) * 512],
                                start=(bias_last and kc == 0),
                                stop=(not bias_last and kc == KC - 1))
                    if bias_last:
                        for m in ms:
                            nc.tensor.matmul(psums[m][:], ones[:1, :128],
                                             bias[:1, n * 512:(n + 1) * 512],
                                             start=False, stop=True)
                    for m in ms:
                        mn = tmp_pool.tile([128, 512], f32, tag="mn",
                                           name="mn")
                        ex = tmp_pool.tile([128, 512], f32, tag="ex",
                                           name="ex")
                        nc.vector.tensor_scalar_min(mn[:], psums[m][:], 0.0)
                        nc.scalar.activation(ex[:], mn[:], AFT.Exp)
                        nc.vector.scalar_tensor_tensor(
                            outs[m][n][:], psums[m][:], 0.0, ex[:],
                            op0=mybir.AluOpType.max,
                            op1=mybir.AluOpType.add)

            def s_partial(b):
                """S partial for batch b, packed as head-pair blocks:
                S_ps[g][:, i*128:+128] = [k_h0|k_h1]^T [q_h0|q_h1] for the
                pair t = g*4+i (h0=2t). Half the matmuls of per-head S; the
                off-diagonal quarters are waste, the diagonal quarters are
                extracted by strided copies in launch_ar."""
                S_ps = [ps_pool.tile([128, 512], f32, tag="pp", name="S_ps")
                        for _ in range(2)]
                for g in range(2):
                    for i in range(4):
                        c0 = 2 * i * 64
                        for lc in range(NT // B):
                            m = b * (NT // B) + lc
                            nc.tensor.matmul(
                                S_ps[g][:, i * 128:(i + 1) * 128],
                                k_t[m][g][:, c0:c0 + 128],
                                q_t[m][g][:, c0:c0 + 128],
                                start=(lc == 0), stop=(lc == NT // B - 1))
                return S_ps

            def launch_ar(b, S_ps):
                """Extract diagonal quarters (strided), fire the AllReduce.
                ccst rows 0:64 = even heads' S, rows 64:128 = odd heads'."""
                ccst = small_pool.tile([128, 512], bf16, tag=f"ccst{b}",
                                       name="ccst")
                for g in range(2):
                    for j in range(2):
                        src = S_ps[g][j * 64:(j + 1) * 64, :].rearrange(
                            "p (i c) -> p i c", i=4)[:, :, j * 64:(j + 1) * 64]
                        dst = ccst[j * 64:(j + 1) * 64,
                                   g * 256:(g + 1) * 256].rearrange(
                            "p (i d) -> p i d", i=4)
                        # Act engine: its queue drains earlier than DVE's
                        # (which is still chewing the k copyouts), so the
                        # AllReduce fires sooner
                        nc.scalar.activation(dst, src, AFT.Copy)
                cc_in = dram_pool.tile([128, 512], bf16, tag=f"ccin{b}",
                                       name="ccin")
                cc_out = dram_pool.tile([128, 512], bf16, tag=f"ccout{b}",
                                        name="ccout")
                nc.sync.dma_start(cc_in[:], ccst[:])
                nc.gpsimd.collective_compute(
                    "AllReduce", mybir.AluOpType.add,
                    replica_groups=[list(range(N_CORES))],
                    ins=[cc_in[:].opt()], outs=[cc_out[:].opt()])
                return cc_out

            bias_q = load_bias("bq")
            bias_k = load_bias("bk")
            xt_q, w_q = load_big(qT_d, xt_pool, "xtA", nsplit=8,
                                 interleave=(W_d["Wq"], w_pool, "wA"))
            xt_k, w_k = load_big(kT_d, xt_pool, "xtB", nsplit=8,
                                 interleave=(W_d["Wk"], w_pool, "wB"))

            # q/k feature tiles are split per n-half: the S matmuls for
            # head group g read only half g, and per-half tiles avoid a
            # false wait on the other half's ELU chain (dep tracking is
            # tile-granular)
            q_t = [[act_pool.tile([128, 512], bf16, tag=f"q{m}h{h}",
                                  name=f"q{m}h{h}") for h in range(2)]
                   for m in range(NT)]
            k_t = [[act_pool.tile([128, 512], bf16, tag=f"k{m}h{h}",
                                  name=f"k{m}h{h}") for h in range(2)]
                   for m in range(NT)]

            # batch 0: project, S partial, fire AllReduce 0 early
            proj_half(xt_q, w_q, bias_q, q_t, 0)
            proj_half(xt_k, w_k, bias_k, k_t, 0)
            cc0 = launch_ar(0, s_partial(0))

            # vT/out-proj loads: after the cc_in(0) DMA (SP is in-order; the
            # xt_v WAR wait on xtA would otherwise delay the collective),
            # before cc_in(1) so they issue as soon as the q tiles free.
            xt_v = load_big(vT_d, xt_pool, "xtA", nsplit=2)
            w_v = load_big(W_d["Wv"], w_pool, "wC", nsplit=2)
            bvT = small_pool.tile([128, KC], f32, tag="bvT", name="bvT")
            nc.sync.dma_start(bvT[:], bvT_d)
            w_o = load_big(W_d["Wo"], w_pool, "wA", nsplit=2)
            bias_o = load_bias("bo")

            # batch 1: project, S partial, AllReduce 1 (queues behind AR 0)
            proj_half(xt_q, w_q, bias_q, q_t, 1)
            proj_half(xt_k, w_k, bias_k, k_t, 1)
            cc1 = launch_ar(1, s_partial(1))

            # ---- denominators: den[m][l, h] = sum_d q*k on DVE (emitted
            # after the S staging so the AllReduces are not stuck behind
            # them in the in-order DVE queue) ----
            dens = []
            for m in range(NT):
                den = tmp_pool.tile([128, 16], f32, tag="den", name="den",
                                    bufs=NT)
                for half in range(2):
                    prod = tmp_pool.tile([128, 512], bf16, tag="prod",
                                         name="prod")
                    nc.vector.tensor_mul(
                        prod[:], q_t[m][half][:], k_t[m][half][:])
                    nc.vector.reduce_sum(
                        den[:, half * 8:(half + 1) * 8],
                        prod[:].rearrange("p (h d) -> p h d", h=8),
                        axis=mybir.AxisListType.X)
                dens.append(den)

            # ---- vT projection overlaps the AllReduces ----
            recipT = small_pool.tile([16, ROWS], f32, tag="recipT",
                                     name="recipT")
            recipT_r = small_pool.tile([16, ROWS], bf16, tag="recipTr",
                                       name="recipTr")
            rbs = [act_pool.tile([128, ROWS], bf16, tag=f"rb{t}",
                                 name=f"rb{t}") for t in range(KC)]

            def dent_half(b):
                # transpose dens -> recipT cols, reciprocal, bf16 copy
                for m in range(b * 4, b * 4 + 4):
                    dent = ps_pool.tile([16, 128], f32, tag="pp",
                                        name="dent")
                    nc.tensor.transpose(dent[:], dens[m][:], ident[:])
                    nc.vector.tensor_scalar_add(
                        recipT[:, m * 128:(m + 1) * 128], dent[:], 1e-6)
                sl = slice(b * 512, (b + 1) * 512)
                nc.vector.reciprocal(recipT[:, sl], recipT[:, sl])
                nc.vector.tensor_copy(recipT_r[:, sl], recipT[:, sl])

            def rb_half(b):
                # rb[t][p, b-half] = recip[2t + (p>=64)] via selector matmul
                for t in range(KC):
                    psr = ps_pool.tile([128, 512], f32, tag="pp", name="psr")
                    nc.tensor.matmul(psr[:], Et[:, t * 128:(t + 1) * 128],
                                     recipT_r[:, b * 512:(b + 1) * 512],
                                     start=True, stop=True)
                    nc.scalar.activation(rbs[t][:, b * 512:(b + 1) * 512],
                                         psr[:], AFT.Copy)

            vTs = [act_pool.tile([128, ROWS], bf16, tag=f"vt{t}",
                                 name=f"vt{t}")
                   for t in range(KC)]
            for t in range(KC):
                ps2 = [ps_pool.tile([128, 512], f32, tag="pp", name="pp")
                       for _ in range(2)]
                for kc in range(KC):
                    for n in range(2):
                        nc.tensor.matmul(
                            ps2[n][:],
                            w_v[:, kc * 1024 + t * 128:
                                kc * 1024 + (t + 1) * 128],
                            xt_v[:, kc * 1024 + n * 512:
                                 kc * 1024 + (n + 1) * 512],
                            start=(kc == 0), stop=(kc == KC - 1))
                if t == 3:
                    dent_half(0)
                elif t == 5:
                    dent_half(1)
                elif t == 6:
                    rb_half(0)
                for n in range(2):
                    # bias bv is per-partition here (rows = dm): fuse it
                    # into the copyout on the Act engine
                    nc.scalar.activation(
                        vTs[t][:, n * 512:(n + 1) * 512], ps2[n][:],
                        AFT.Identity, bias=bvT[:, t:t + 1])

            # ---- per batch: reload reduced S, attnT, output projection ----
            attnT = [act_pool.tile([128, ROWS], bf16, tag=f"at{t}",
                                   name=f"attnT{t}")
                     for t in range(KC)]

            def ccj_load(cc_out, b):
                ccJ0 = small_pool.tile([64, 512], bf16, tag=f"ccJ0{b}",
                                       name="ccJ0")
                ccJ1 = small_pool.tile([128, 512], bf16, tag=f"ccJ1{b}",
                                       name="ccJ1")
                nc.sync.dma_start(ccJ0[0:64, :], cc_out[0:64, :])
                nc.sync.dma_start(ccJ1[64:128, :], cc_out[64:128, :])
                return ccJ0, ccJ1

            def attn_half(ccJs, b):
                for t in range(KC):
                    ps = ps_pool.tile([128, 512], f32, tag="pp", name="pa")
                    for j in range(2):
                        col = (t // 4) * 256 + (t % 4) * 64
                        nc.tensor.matmul(
                            ps[j * 64:(j + 1) * 64, :],
                            ccJs[j][j * 64:(j + 1) * 64, col:col + 64],
                            vTs[t][j * 64:(j + 1) * 64,
                                   b * RPB:(b + 1) * RPB],
                            start=True, stop=True)
                    nc.vector.tensor_mul(
                        attnT[t][:, b * RPB:(b + 1) * RPB], ps[:],
                        rbs[t][:, b * RPB:(b + 1) * RPB])

            def out_half(mh, mid_hook=None, taper=False):
                base = mh * 4
                for n in range(2):
                    if n == 1 and mid_hook is not None:
                        mid_hook()
                    # taper the very last groups (2,1,1) so the drain tail
                    # is one short copyout+store deep instead of four
                    if taper and n == 1:
                        grps = [[base], [base + 1], [base + 2], [base + 3]]
                    else:
                        grps = [[base + i for i in range(4)]]
                    for ms in grps:
                        psums = {m: ps_pool.tile([128, 512], f32, tag="pp",
                                                 name="pp") for m in ms}
                        for m in ms:
                            nc.tensor.matmul(psums[m][:], ones[:1, :128],
                                             bias_o[:1,
                                                    n * 512:(n + 1) * 512],
                                             start=True, stop=False)
                        for kc in range(KC):
                            for m in ms:
                                nc.tensor.matmul(
                                    psums[m][:],
                                    attnT[kc][:, m * 128:(m + 1) * 128],
                                    w_o[:, kc * 1024 + n * 512:
                                        kc * 1024 + (n + 1) * 512],
                                    start=False, stop=(kc == KC - 1))
                        for m in ms:
                            ot = tmp_pool.tile([128, 512], f32,
                                               tag=("mn" if m % 2 else "ex"),
                                               name="ot")
                            if m % 2:
                                nc.scalar.activation(ot[:], psums[m][:],
                                                     AFT.Copy)
                            else:
                                nc.vector.tensor_copy(ot[:], psums[m][:])
                            nc.sync.dma_start(
                                out_d[m * 128:(m + 1) * 128,
                                      n * 512:(n + 1) * 512], ot[:])

            ccJs0 = ccj_load(cc0, 0)
            attn_half(ccJs0, 0)
            rb_half(1)
            # ccJ(1) DMAs are emitted between out(0)'s two store groups:
            # SP is in-order, so putting them after all out(0) stores would
            # delay them to ~the last store, stalling attnT(1); putting them
            # before would park SP on the AllReduce-1 semaphore and stall
            # the early stores instead.
            ccJs1 = []
            out_half(0, mid_hook=lambda: ccJs1.extend(ccj_load(cc1, 1)))
            attn_half(ccJs1, 1)
            out_half(1, taper=True)

    nc.compile()
    return nc


def _get_nc():
    if "nc" not in _CACHE:
        _CACHE["nc"] = _build()
    return _CACHE["nc"]


def _make_econst():
    E = np.zeros((16, DM), np.float32)
    for t in range(KC):
        E[2 * t, t * 128:t * 128 + 64] = 1.0
        E[2 * t + 1, t * 128 + 64:(t + 1) * 128] = 1.0
    return E


def kernel(query, key, value, Wq, bq, Wk, bk, Wv, bv, Wo, bo, **kw):
    from concourse.bass_utils import run_bass_kernel_spmd

    nc = _get_nc()
    bf = ml_dtypes.bfloat16
    query = np.asarray(query, dtype=np.float32)
    key = np.asarray(key, dtype=np.float32)
    value = np.asarray(value, dtype=np.float32)
    weights = {n: np.ascontiguousarray(np.asarray(w, np.float32).astype(bf))
               for n, w in (("Wq", Wq), ("Wk", Wk), ("Wv", Wv), ("Wo", Wo))}
    biases = {n: np.ascontiguousarray(
                  np.asarray(b, np.float32).reshape(1, DM).astype(bf))
              for n, b in (("bq", bq), ("bk", bk), ("bo", bo))}
    biases["bvT"] = np.ascontiguousarray(
        np.asarray(bv, np.float32).reshape(KC, 128).T)
    econst = _make_econst()

    in_maps = []
    for c in range(N_CORES):
        sl = slice(c * RPB, (c + 1) * RPB)
        m = {
            "qT": np.ascontiguousarray(
                np.concatenate([query[b, sl] for b in range(B)], 0).T
            ).astype(bf),
            "kT": np.ascontiguousarray(
                np.concatenate([key[b, sl] for b in range(B)], 0).T
            ).astype(bf),
            "vT": np.ascontiguousarray(
                np.concatenate([value[b, sl] for b in range(B)], 0).T
            ).astype(bf),
            "Econst": econst.astype(bf),
        }
        m.update(weights)
        m.update(biases)
        in_maps.append(m)

    res = run_bass_kernel_spmd(nc, in_maps, list(range(N_CORES)), **kw)
    out = np.empty((B, L, DM), np.float32)
    for c in range(N_CORES):
        o = np.asarray(res.results[c]["out"]).astype(np.float32)
        for b in range(B):
            out[b, c * RPB:(c + 1) * RPB] = o[b * RPB:(b + 1) * RPB]
    if kw:
        return out, res
    return out
